# revision 1
# baseline (speedup 1.0000x reference)
"""GAT layer (nn_GATLayer) on 8 Trainium2 NeuronCores.

Math (reference):
    Wh = X @ weight                      [N, F]
    s  = Wh @ a[:F];  t = Wh @ a[F:]     [N, 1]
    e  = relu(s_i + t_j)                 [N, N]
    att = softmax(where(A > 0, e, -9e15), axis=1)
    out = elu(att @ Wh)

Kernel formulation (shift-free softmax, exact up to fp rounding):
    p_ij  = A_ij * max(exp(s_i + t_j), 1)     (exp(relu(x)) = max(exp(x), 1))
    out_i = elu((p_i: @ Wh) / sum_j p_ij)

Sharding: 1D row partition of A across 8 cores (1024 rows each); X,
weight, a replicated; out rows gathered on host.

Per-core dataflow (v2: transposed orientation [j, i]; the only large
transpose is A's, as 64 big DRAM->SBUF xbar DMAs):
  - A_blk int32 -> bf16 {0,1.0} via SWDGE DRAM->DRAM cast (8 chunks),
    then 64 DmaTranspose [1024, 128] -> at_slab [128 j, 1024 i].
  - X -> bf16 DRAM (D2D cast), 4 big transposes -> X^T chunks; Wh_nat
    [128 j, 128 f] + t columns from PE (stationary = X^T slice, moving
    = [weight | w_t]); w_t = weight.T-chunks @ a via tiny PE matmuls.
  - s (own rows) from an X_own mini-matmul, assembled into a DRAM row,
    broadcast-cast back as S_bcast [128, 1024 i].
  - main loop over 64 j-tiles: ACT z = exp(S_bcast + t_j); one fused
    DVE op p^T = (z max 1) * at_slab; PE: psum_oT [128 f, 1024 i] +=
    contraction of Wh_nat[jt] with p^T (N=512 x2), denominator row via
    ones stationary.
  - epilogue: reciprocal of denom -> DRAM broadcast -> scale, ELU
    (exp(min(x,0)) - 1 + max(x,0)), 8 PE transposes back to natural,
    DMA out.
"""

import numpy as np

import concourse.bass as bass
import concourse.bacc as bacc
import concourse.mybir as mybir
import concourse.tile as tile
from concourse.bass_utils import run_bass_kernel_spmd

N = 8192
F_IN = 512
F_OUT = 128
N_CORES = 8
ROWS = N // N_CORES          # 1024 rows per core
RT = ROWS // 128             # 8 own row tiles
NT = N // 128                # 64 j tiles
KC = F_IN // 128             # 4 f_in chunks
DCH = 1                      # A cast D2D chunks

FP32 = mybir.dt.float32
BF16 = mybir.dt.bfloat16
Alu = mybir.AluOpType
Act = mybir.ActivationFunctionType

_cache = {}


def _build(repeat=1):
    nc = bacc.Bacc("TRN2", target_bir_lowering=False, debug=False,
                   num_devices=N_CORES)

    A_blk = nc.dram_tensor("A_blk", [ROWS, N], mybir.dt.int32, kind="ExternalInput")
    X_own = nc.dram_tensor("X_own", [ROWS, F_IN], FP32, kind="ExternalInput")
    weight = nc.dram_tensor("weight", [F_IN, F_OUT], FP32, kind="ExternalInput")
    a_vec = nc.dram_tensor("a_vec", [2 * F_OUT, 1], FP32, kind="ExternalInput")
    ident = nc.dram_tensor("ident", [128, 128], FP32, kind="ExternalInput")
    out_d = nc.dram_tensor("out", [ROWS, F_OUT], FP32, kind="ExternalOutput")

    with tile.TileContext(nc) as tc:
        for rep in range(repeat):
            _body(nc, tc, rep, A_blk, X_own, weight, a_vec, ident, out_d)

    nc.compile()
    return nc


def _body(nc, tc, rep, A_blk, X_own, weight, a_vec, ident, out_d):
    with tc.tile_pool(name=f"dram{rep}", bufs=1, space="DRAM") as dram_pool:
            A_bf = dram_pool.tile([ROWS, N], BF16)
            Xo_bf = dram_pool.tile([ROWS, F_IN], BF16)
            s_dram = dram_pool.tile([1, ROWS], FP32)
            r_dram = dram_pool.tile([1, ROWS], FP32)
            CCR = ROWS + 2 * RT  # wh rows + t rows (f32 as 2x bf16 rows)
            cc_in_m = nc.dram_tensor(f"cc_in_m{rep}", [CCR, F_OUT], BF16)
            cc_out_m = nc.dram_tensor(
                f"cc_out_m{rep}", [N_CORES * CCR, F_OUT], BF16,
                addr_space="Shared",
            )

            # ---- D2D casts (SWDGE): A int32 -> bf16, X f32 -> bf16 ----
            for c in range(DCH):
                w = N // DCH
                nc.gpsimd.dma_start(
                    out=A_bf[:, w * c : w * (c + 1)],
                    in_=A_blk[:, w * c : w * (c + 1)],
                )
            nc.gpsimd.dma_start(out=Xo_bf[:, :], in_=X_own[:, :])

            with (
                tc.tile_pool(name=f"setup{rep}", bufs=1) as setup,
                tc.tile_pool(name=f"slab{rep}", bufs=12) as slab_pool,
                tc.tile_pool(name=f"zz{rep}", bufs=6) as zz_pool,
                tc.tile_pool(name=f"pp{rep}", bufs=6) as pp_pool,
                tc.tile_pool(name=f"epi{rep}", bufs=2) as epi_pool,
                tc.tile_pool(name=f"psA{rep}", bufs=2, space="PSUM") as psA,
            ):
                # ---------------- setup ----------------
                idn = setup.tile([128, 128], FP32)
                nc.sync.dma_start(out=idn, in_=ident[:, :])
                ones_c = setup.tile([128, 1], BF16)
                nc.vector.memset(ones_c, 1.0)

                a_cat = setup.tile([128, 2], BF16)
                nc.gpsimd.dma_start(out=a_cat[:, 0:1], in_=a_vec[0:F_OUT, :])
                nc.gpsimd.dma_start(out=a_cat[:, 1:2], in_=a_vec[F_OUT:, :])

                # w_all[k] = [weight_k bf16 | w_t_k | w_s_k]  [128, 130]
                w_all = []
                for k in range(KC):
                    wa = setup.tile([128, F_OUT + 2], BF16, tag=f"w_all{k}")
                    nc.gpsimd.dma_start(
                        out=wa[:, 0:F_OUT], in_=weight[128 * k : 128 * (k + 1), :]
                    )
                    w_all.append(wa)
                for k in range(KC):
                    wT = setup.tile([128, 128], BF16, tag=f"wT{k}")
                    nc.sync.dma_start(
                        out=wT, in_=w_all[k][:, 0:F_OUT], transpose=True
                    )
                    ps = psA.tile([128, 2], FP32, tag="ps")
                    nc.tensor.matmul(ps, wT, a_cat, start=True, stop=True)
                    # col F_OUT = w_t (a[F:]), col F_OUT+1 = w_s (a[:F])
                    nc.vector.tensor_copy(
                        w_all[k][:, F_OUT : F_OUT + 1], ps[:, 1:2]
                    )
                    nc.vector.tensor_copy(
                        w_all[k][:, F_OUT + 1 : F_OUT + 2], ps[:, 0:1]
                    )

                # ---------------- X_own^T chunks ----------------
                xoT = []
                for k in range(KC):
                    xt = setup.tile([128, ROWS], BF16, tag=f"xoT{k}")
                    nc.sync.dma_start(
                        out=xt, in_=Xo_bf[:, 128 * k : 128 * (k + 1)], transpose=True
                    )
                    xoT.append(xt)

                # ------- own rows: [Wh | t | s] = Xo^T.T-contract @ w_all -------
                s_cols = setup.tile([128, RT], FP32)
                t_own = setup.tile([128, RT], FP32)
                for q in range(RT):
                    ps = psA.tile([128, F_OUT + 2], FP32, tag="ps")
                    for k in range(KC):
                        nc.tensor.matmul(
                            ps,
                            xoT[k][:, 128 * q : 128 * (q + 1)],
                            w_all[k],
                            start=(k == 0),
                            stop=(k == KC - 1),
                        )
                    wh = zz_pool.tile([128, F_OUT], BF16, tag="wh_own")
                    nc.vector.tensor_copy(wh, ps[:, 0:F_OUT])
                    nc.sync.dma_start(
                        out=cc_in_m[128 * q : 128 * (q + 1), :], in_=wh
                    )
                    nc.vector.tensor_copy(
                        t_own[:, q : q + 1], ps[:, F_OUT : F_OUT + 1]
                    )
                    nc.vector.tensor_copy(
                        s_cols[:, q : q + 1], ps[:, F_OUT + 1 : F_OUT + 2]
                    )
                # s -> DRAM row -> broadcast
                ps_sT = psA.tile([RT, 128], FP32, tag="ps")
                nc.tensor.transpose(ps_sT, s_cols, idn)
                sT = setup.tile([RT, 128], FP32)
                nc.vector.tensor_copy(sT, ps_sT)
                nc.sync.dma_start(out=s_dram[:, :], in_=sT)
                s_bc = setup.tile([128, ROWS], FP32)
                nc.gpsimd.dma_start(
                    out=s_bc,
                    in_=bass.AP(
                        tensor=s_dram.tensor, offset=s_dram.offset,
                        ap=[[0, 128], [1, ROWS]],
                    ),
                )
                # t-own -> [RT, 128] tile-major, f32 bitcast into bf16 rows
                ps_tT = psA.tile([RT, 128], FP32, tag="ps")
                nc.tensor.transpose(ps_tT, t_own, idn)
                tT = setup.tile([RT, 128], FP32)
                nc.vector.tensor_copy(tT, ps_tT)
                nc.sync.dma_start(
                    out=cc_in_m[ROWS : ROWS + 2 * RT, :],
                    in_=tT.bitcast(BF16),
                )

                # ------- single AllGather (Wh | t) across cores -------
                nc.gpsimd.collective_compute(
                    "AllGather", Alu.bypass,
                    replica_groups=[list(range(N_CORES))],
                    ins=[cc_in_m[:, :]], outs=[cc_out_m[:, :]],
                )
                # one big DMA for all Wh tiles: [128, NT, F_OUT]
                wh_all = setup.tile([128, NT, F_OUT], BF16)
                for c in range(N_CORES):
                    nc.sync.dma_start(
                        out=wh_all[:, RT * c : RT * (c + 1), :],
                        in_=cc_out_m[CCR * c : CCR * c + ROWS, :].rearrange(
                            "(r p) f -> p r f", p=128
                        ),
                    )
                wh_nat = [wh_all[:, r, :] for r in range(NT)]
                # t blocks: per core, 2*RT bf16 rows = [RT, 128] f32
                tg = setup.tile([NT, 128], FP32)
                for c in range(N_CORES):
                    nc.sync.dma_start(
                        out=tg[RT * c : RT * (c + 1), :].bitcast(BF16),
                        in_=cc_out_m[CCR * c + ROWS : CCR * c + ROWS + 2 * RT, :],
                    )
                ps_tc = psA.tile([128, NT], FP32, tag="ps")
                nc.tensor.transpose(ps_tc, tg, idn[0:NT, 0:NT])
                t_cols = setup.tile([128, NT], FP32)
                nc.vector.tensor_copy(t_cols, ps_tc)

                # ---------------- A^T slabs + main loop ----------------
                with (
                    tc.tile_pool(name=f"psO{rep}", bufs=1, space="PSUM") as psO,
                    tc.tile_pool(name=f"psD{rep}", bufs=1, space="PSUM") as psD,
                ):
                    ps_oT = psO.tile([128, ROWS], FP32)
                    ps_d = psD.tile([1, ROWS], FP32)
                    for jt in range(NT):
                        at = slab_pool.tile([128, ROWS], BF16)
                        nc.sync.dma_start(
                            out=at, in_=A_bf[:, 128 * jt : 128 * (jt + 1)],
                            transpose=True,
                        )
                        z = zz_pool.tile([128, ROWS], BF16)
                        nc.scalar.activation(
                            out=z, in_=s_bc, func=Act.Exp,
                            bias=t_cols[:, jt : jt + 1],
                        )
                        p = pp_pool.tile([128, ROWS], BF16)
                        nc.vector.scalar_tensor_tensor(
                            out=p, in0=z, scalar=1.0, in1=at,
                            op0=Alu.max, op1=Alu.mult,
                        )
                        first, last = jt == 0, jt == NT - 1
                        for h in range(2):
                            sl = slice(512 * h, 512 * (h + 1))
                            nc.tensor.matmul(
                                ps_oT[:, sl], wh_nat[jt], p[:, sl],
                                start=first, stop=last, skip_group_check=True,
                            )
                            nc.tensor.matmul(
                                ps_d[:, sl], ones_c, p[:, sl],
                                start=first, stop=last, skip_group_check=True,
                            )

                    # ---------------- epilogue ----------------
                    rec = epi_pool.tile([1, ROWS], FP32, tag="rec")
                    nc.vector.reciprocal(rec, ps_d)
                    # broadcast rec along partitions via K=1 outer product
                    ones_r = epi_pool.tile([1, 128], FP32, tag="ones_r")
                    nc.vector.memset(ones_r, 1.0)
                    ps_rb = psD.tile([128, ROWS], FP32, tag="ps_rb")
                    for h in range(2):
                        sl = slice(512 * h, 512 * (h + 1))
                        nc.tensor.matmul(
                            ps_rb[:, sl], ones_r, rec[:, sl],
                            start=True, stop=True,
                        )
                    rb_sb = epi_pool.tile([128, ROWS], FP32, tag="rb_sb")
                    nc.scalar.copy(rb_sb, ps_rb)
                    xsc = epi_pool.tile([128, ROWS], FP32, tag="xsc")
                    nc.vector.tensor_tensor(
                        out=xsc, in0=ps_oT, in1=rb_sb, op=Alu.mult
                    )
                    # ELU = exp(min(x,0)) - 1 + max(x,0)
                    m0 = epi_pool.tile([128, ROWS], FP32, tag="m0")
                    nc.vector.tensor_scalar(
                        out=m0, in0=xsc, scalar1=0.0, scalar2=None, op0=Alu.min
                    )
                    e0 = epi_pool.tile([128, ROWS], FP32, tag="e0")
                    nc.scalar.activation(out=e0, in_=m0, func=Act.Exp)
                    r0 = epi_pool.tile([128, ROWS], FP32, tag="r0")
                    nc.vector.tensor_scalar(
                        out=r0, in0=xsc, scalar1=0.0, scalar2=None, op0=Alu.max
                    )
                    oT = epi_pool.tile([128, ROWS], FP32, tag="oT")
                    nc.vector.scalar_tensor_tensor(
                        out=oT, in0=e0, scalar=-1.0, in1=r0,
                        op0=Alu.add, op1=Alu.add,
                    )
                    for q in range(RT):
                        ps_f = psA.tile([128, 128], FP32, tag="ps")
                        nc.tensor.transpose(
                            ps_f, oT[:, 128 * q : 128 * (q + 1)], idn
                        )
                        of = epi_pool.tile([128, F_OUT], FP32, tag="of")
                        nc.scalar.copy(of, ps_f)
                        nc.sync.dma_start(
                            out=out_d[128 * q : 128 * (q + 1), :], in_=of
                        )


def kernel(X, A, weight, a, _trace=False, _tmpdir=None):
    X = np.ascontiguousarray(np.asarray(X, dtype=np.float32))
    A = np.ascontiguousarray(np.asarray(A, dtype=np.int32))
    weight = np.ascontiguousarray(np.asarray(weight, dtype=np.float32))
    a = np.ascontiguousarray(np.asarray(a, dtype=np.float32))

    if "nc" not in _cache:
        _cache["nc"] = _build()
    nc = _cache["nc"]

    ident = np.eye(128, dtype=np.float32)
    in_maps = []
    for c in range(N_CORES):
        i0 = c * ROWS
        in_maps.append(
            {
                "A_blk": A[i0 : i0 + ROWS],
                "X_own": X[i0 : i0 + ROWS],
                "weight": weight,
                "a_vec": a,
                "ident": ident,
            }
        )

    res = run_bass_kernel_spmd(
        nc, in_maps, core_ids=list(range(N_CORES)), trace=_trace, tmpdir=_tmpdir
    )
    out = np.concatenate([res.results[c]["out"] for c in range(N_CORES)], axis=0)
    if _trace:
        kernel._last_results = res
    return out



# revision 34
# speedup vs baseline: 2.5443x; 2.5443x over previous
"""GAT layer (nn_GATLayer) on 8 Trainium2 NeuronCores.

Math (reference):
    Wh = X @ weight                      [N, F]
    s  = Wh @ a[:F];  t = Wh @ a[F:]     [N, 1]
    e  = relu(s_i + t_j)                 [N, N]
    att = softmax(where(A > 0, e, -9e15), axis=1)
    out = elu(att @ Wh)

Kernel formulation (shift-free softmax, exact up to fp rounding):
    p_ij  = A_ij * max(exp(s_i) * exp(t_j), 1)   (exp(relu(x)) = max(exp(x), 1))
    out_i = elu((p_i: @ Wh) / sum_j p_ij)
A global scale c (=1/4) keeps all fp8 operands in e4m3 normal range:
the host mask carries {0, c}, z' = exp(s - ln(1/c)) * exp(t), and every
psum contribution is uniformly c^2-scaled, which cancels in num/den.

Sharding: 1D row partition across 8 cores (1024 rows each). Host-side
prep is layout/dtype only: X^T (bf16, grouped, rotated so group 0 is
own rows), A^T slab per core (fp8 {0, c}, same group rotation), weight
bf16. All model math (Wh, s, t, exp, softmax, aggregation, ELU) runs
on device.

Per-core dataflow, all in [j (partition), i (free)] orientation:
  - setup: w_all[k] = [weight_k | w*a2 | w*a1]; per j-tile pair one PE
    pass gives [Wh | t] (fp8 Wh); s from group-0 X^T; es/s broadcast
    rows via K=1 PE outer products; t transposed to a row for exp.
  - main loop over 32 j-tile pairs, three z sources balanced across
    engines (PE rank-1 outer product into bf16 psum / DVE 4x ptr-mult
    / ACT exp with bias ptr); two mask paths:
      D-pairs: p8 = (c max z) * mask in one DVE op -> 2 DoubleRow
        matmuls (numerator [128 f, 1024 i], denominator [1, 1024]);
      P-pairs: mask-term DR matmuls consume the raw fp8 mask, the
        relu-term r8 = Pool tensor_tensor mult of relu(z - c).
  - epilogue: den -> columns via K=1 matmuls, reciprocal, PE
    transposes to natural [i, f], fused scale+ELU, one output DMA.
"""

import numpy as np
import ml_dtypes

import concourse.bass as bass
import concourse.bacc as bacc
import concourse.mybir as mybir
import concourse.tile as tile
from concourse.bass_utils import run_bass_kernel_spmd

N = 8192
F_IN = 512
F_OUT = 128
N_CORES = 8
ROWS = N // N_CORES          # 1024 rows per core
NT = N // 128                # 64 j tiles
NP = NT // 2                 # 32 j tile pairs
KC = F_IN // 128             # 4 f_in chunks
NG = 8                       # XT groups (8 j-tiles each)

SCALE = 0.25                 # global fp8 range scale c
LNS = float(np.log(1.0 / SCALE))

FP32 = mybir.dt.float32
BF16 = mybir.dt.bfloat16
FP8 = mybir.dt.float8e4
Alu = mybir.AluOpType
Act = mybir.ActivationFunctionType
DR = mybir.MatmulPerfMode.DoubleRow

# --- engine lane tables (tuned against the TimelineSim cost model) ---
# P_PAIRS: pairs routed through the relu decomposition (Pool mask-mult)
P_PAIRS = frozenset({1, 3, 5, 6, 9, 11, 14, 17, 19, 21, 22, 25, 27, 30})
# z source per pair: 'pe' (rank-1 matmul into psum, D-pairs only),
# 'act' (exp), 'dve' (ptr-mult)
_ZSRC = {}
for _t in range(NP):
    _ZSRC[_t] = "dve" if _t in P_PAIRS else "act"
# relu engine for P-pairs: DVE 4x for most, ACT for some
_RELU_ACT = frozenset({3, 11, 19, 27})

_cache = {}


def _build():
    nc = bacc.Bacc("TRN2", target_bir_lowering=False, debug=False,
                   num_devices=N_CORES)

    XTg = nc.dram_tensor("XTg", [NG, KC, 128, ROWS], BF16, kind="ExternalInput")
    AT8 = nc.dram_tensor("AT8", [N, ROWS], FP8, kind="ExternalInput")
    w_in = nc.dram_tensor("w_in", [F_IN, F_OUT], BF16, kind="ExternalInput")
    a_vec = nc.dram_tensor("a_vec", [2 * F_OUT, 1], FP32, kind="ExternalInput")
    ident = nc.dram_tensor("ident", [128, 128], FP32, kind="ExternalInput")
    out_d = nc.dram_tensor("out", [ROWS, F_OUT], FP32, kind="ExternalOutput")

    with tile.TileContext(nc) as tc:
        _body(nc, tc, XTg, AT8, w_in, a_vec, ident, out_d)

    nc.compile()
    return nc


def _body(nc, tc, XTg, AT8, w_in, a_vec, ident, out_d):
    with (
        tc.tile_pool(name="setup", bufs=1) as setup,
        tc.tile_pool(name="xtg", bufs=2) as xtg_pool,
        tc.tile_pool(name="at", bufs=1) as at_pool,
        tc.tile_pool(name="zz", bufs=3) as zz_pool,
        tc.tile_pool(name="pp", bufs=3) as pp_pool,
        tc.tile_pool(name="epi", bufs=1) as epi,
    ):
        setup_psum = tc.tile_pool(name="psA", bufs=1, space="PSUM")
        psA = setup_psum.__enter__()
        # ---------------- setup: weights ----------------
        idn = setup.tile([128, 128], FP32)
        nc.sync.dma_start(out=idn, in_=ident[:, :])
        idn_bf = setup.tile([128, 128], BF16)
        nc.vector.tensor_copy(idn_bf, idn)
        w_sb = setup.tile([128, KC, 128], BF16)
        nc.sync.dma_start(
            out=w_sb, in_=w_in.rearrange("(k p) f -> p k f", p=128)
        )
        a_cat = setup.tile([128, 2], BF16)
        nc.gpsimd.dma_start(
            out=a_cat, in_=a_vec.rearrange("(h p) o -> p (h o)", p=128)
        )

        # w_all[k] = [weight_k | w_t_k | w_s_k]  [128, 130]
        w_all = []
        for k in range(KC):
            wa = setup.tile([128, F_OUT + 2], BF16, tag=f"w_all{k}")
            nc.vector.tensor_copy(wa[:, 0:F_OUT], w_sb[:, k, :])
            ps_wT = psA.tile([128, 128], BF16, tag="wT")
            nc.tensor.transpose(ps_wT, w_sb[:, k, :], idn_bf)
            wT = setup.tile([128, 128], BF16, tag=f"wT{k}")
            nc.vector.tensor_copy(wT, ps_wT)
            ps_a = psA.tile([128, 2], FP32, tag="pa")
            nc.tensor.matmul(ps_a, wT, a_cat, start=True, stop=True)
            # col F_OUT = w_t (a[F:]), col F_OUT+1 = w_s (a[:F])
            nc.vector.tensor_copy(wa[:, F_OUT : F_OUT + 1], ps_a[:, 1:2])
            nc.vector.tensor_copy(wa[:, F_OUT + 1 : F_OUT + 2], ps_a[:, 0:1])
            w_all.append(wa)

        # first XT group doubles as own-rows X^T (host rotates groups)
        xtg0 = xtg_pool.tile([128, KC, ROWS], BF16, tag="g0")
        nc.sync.dma_start(out=xtg0, in_=XTg[0].rearrange("k p i -> p k i"))

        # ---------------- s (own rows) + broadcast rows ----------------
        ps_s = psA.tile([1, ROWS], FP32, tag="ps_s")
        for h in range(2):
            sl = slice(512 * h, 512 * (h + 1))
            for k in range(KC):
                nc.tensor.matmul(
                    ps_s[:, sl],
                    w_all[k][:, F_OUT + 1 : F_OUT + 2],
                    xtg0[:, k, sl],
                    start=(k == 0), stop=(k == KC - 1),
                    skip_group_check=True,
                )
        nls1 = setup.tile([1, 1], FP32, tag="nls1")
        nc.vector.memset(nls1, -LNS)
        es_row = setup.tile([1, ROWS], BF16)
        nc.scalar.activation(out=es_row, in_=ps_s, func=Act.Exp, bias=nls1)
        s_row = setup.tile([1, ROWS], BF16)
        nc.scalar.copy(s_row, ps_s)

        ones_r = setup.tile([1, 128], BF16)
        nc.vector.memset(ones_r, 1.0)
        es_bc = setup.tile([128, ROWS], BF16)
        s_bc = setup.tile([128, ROWS], BF16)
        for h in range(2):
            sl = slice(512 * h, 512 * (h + 1))
            ps_b = psA.tile([128, 512], FP32, tag="bc")
            nc.tensor.matmul(ps_b, ones_r, es_row[:, sl], start=True, stop=True)
            nc.vector.tensor_copy(es_bc[:, sl], ps_b)
            ps_b2 = psA.tile([128, 512], FP32, tag="bc")
            nc.tensor.matmul(ps_b2, ones_r, s_row[:, sl], start=True, stop=True)
            nc.vector.tensor_copy(s_bc[:, sl], ps_b2)

        ones_c = setup.tile([128, 2, 128], FP8)
        nc.vector.memset(ones_c, 1.0)
        ones1 = setup.tile([1, 1], FP32, tag="ones1")
        nc.vector.memset(ones1, 1.0)

        # ---------------- Wh | t for all j tiles ----------------
        wh_all = setup.tile([128, NT, F_OUT], FP8)
        et_cols = setup.tile([128, NT], FP32)
        ts_cols = setup.tile([128, NT], FP32)
        nsc_c = setup.tile([128, 1], FP32, tag="nsc_c")
        nc.vector.memset(nsc_c, -SCALE)

        setup_psum.__exit__(None, None, None)

        with (
            tc.tile_pool(name="psO", bufs=1, space="PSUM") as psO,
            tc.tile_pool(name="psD", bufs=1, space="PSUM") as psD,
        ):
            ps_oT = psO.tile([128, ROWS], FP32)
            ps_d = psD.tile([128, ROWS], FP32)

            with tc.tile_pool(name="psS", bufs=4, space="PSUM") as psS:
                at_tiles = []

                def emit_setup(g):
                    if g == 0:
                        xtg = xtg0
                    else:
                        xtg = xtg_pool.tile([128, KC, ROWS], BF16)
                        nc.sync.dma_start(
                            out=xtg, in_=XTg[g].rearrange("k p i -> p k i")
                        )
                    at = at_pool.tile([128, 8, ROWS], FP8, tag=f"at{g}")
                    at_tiles.append(at)
                    nc.sync.dma_start(
                        out=at,
                        in_=AT8[ROWS * g : ROWS * (g + 1), :].rearrange(
                            "(t p) i -> p t i", p=128
                        ),
                    )
                    for q in range(4):
                        jt0 = 8 * g + 2 * q
                        ps_p = psS.tile([128, 2, F_OUT + 2], FP32)
                        for v in range(2):
                            co = 128 * (2 * q + v)
                            for k in range(KC):
                                nc.tensor.matmul(
                                    ps_p[:, v, :],
                                    xtg[:, k, co : co + 128],
                                    w_all[k],
                                    start=(k == 0), stop=(k == KC - 1),
                                    skip_group_check=True,
                                )
                        # copies (gpsimd cannot access PSUM)
                        nc.scalar.copy(
                            wh_all[:, jt0 : jt0 + 2, :], ps_p[:, :, 0:F_OUT]
                        )
                        nc.scalar.activation(
                            out=et_cols[:, jt0 : jt0 + 2],
                            in_=ps_p[:, :, F_OUT : F_OUT + 1],
                            func=Act.Exp,
                        )
                        nc.vector.tensor_scalar(
                            out=ts_cols[:, jt0 : jt0 + 2],
                            in0=ps_p[:, :, F_OUT : F_OUT + 1],
                            scalar1=-LNS, scalar2=None, op0=Alu.add,
                        )

                def emit_pair(t):
                    at = at_tiles[t // 4]
                    s0 = 2 * (t % 4)
                    atsl = at[:, s0 : s0 + 2, :]
                    first, last = t == 0, t == NP - 1
                    zp = zz_pool.tile([128, 2, ROWS], BF16)
                    for v in range(2):
                        jt = 2 * t + v
                        if _ZSRC[t] == "act":
                            nc.scalar.activation(
                                out=zp[:, v, :], in_=s_bc, func=Act.Exp,
                                bias=ts_cols[:, jt : jt + 1],
                            )
                        else:
                            nc.vector.tensor_scalar(
                                out=zp[:, v, :], in0=es_bc,
                                scalar1=et_cols[:, jt : jt + 1],
                                scalar2=None, op0=Alu.mult,
                            )
                    if t in P_PAIRS:
                        # clamp on DVE (4x), mask-mult on Pool: the host
                        # mask already carries the scale c
                        rt = pp_pool.tile([128, 2, ROWS], BF16, tag="rt")
                        nc.vector.tensor_scalar(
                            out=rt, in0=zp, scalar1=SCALE, scalar2=None,
                            op0=Alu.max,
                        )
                        pp = pp_pool.tile([128, 2, ROWS], FP8, tag="p8")
                        nc.gpsimd.tensor_tensor(
                            out=pp, in0=rt, in1=atsl, op=Alu.mult
                        )
                    else:
                        pp = pp_pool.tile([128, 2, ROWS], FP8, tag="p8")
                        nc.vector.scalar_tensor_tensor(
                            out=pp, in0=zp, scalar=SCALE,
                            in1=atsl, op0=Alu.max, op1=Alu.mult,
                        )
                    if True:
                        for h in range(2):
                            sl = slice(512 * h, 512 * (h + 1))
                            nc.tensor.matmul(
                                ps_oT[:, sl],
                                wh_all[:, 2 * t : 2 * t + 2, :],
                                pp[:, :, sl], start=first, stop=last,
                                perf_mode=DR, skip_group_check=True,
                            )
                            nc.tensor.matmul(
                                ps_d[:, sl], ones_c, pp[:, :, sl],
                                start=first, stop=last,
                                perf_mode=DR, skip_group_check=True,
                            )

                # software-pipelined emission: setup(g+1) ahead of pairs(g)
                emit_setup(0)
                for g in range(NG):
                    if g + 1 < NG:
                        emit_setup(g + 1)
                    for t in range(4 * g, 4 * g + 4):
                        emit_pair(t)

            # ---------------- epilogue ----------------
            with tc.tile_pool(name="psE", bufs=2, space="PSUM") as psE:
                den_row = epi.tile([1, ROWS], FP32, tag="den")
                nc.scalar.copy(den_row, ps_d[0:1, :])
                ps_dc = psE.tile([128, 8], FP32, tag="dc")
                for q in range(8):
                    nc.tensor.matmul(
                        ps_dc[:, q : q + 1],
                        den_row[:, 128 * q : 128 * (q + 1)], ones1,
                        start=True, stop=True, skip_group_check=True,
                    )
                rec_cols = epi.tile([128, 8], FP32, tag="rec")
                nc.vector.reciprocal(rec_cols, ps_dc)
                num_sb = epi.tile([128, ROWS], FP32, tag="num")
                nc.scalar.copy(num_sb, ps_oT)
                of_all = epi.tile([128, 8, F_OUT], FP32, tag="of")
                for q in range(8):
                    ps_f = psE.tile([128, 128], FP32, tag="f")
                    nc.tensor.transpose(
                        ps_f, num_sb[:, 128 * q : 128 * (q + 1)], idn
                    )
                    m0 = epi.tile([128, 128], FP32, tag=f"m0_{q % 2}")
                    nc.vector.tensor_scalar(
                        out=m0, in0=ps_f, scalar1=rec_cols[:, q : q + 1],
                        scalar2=0.0, op0=Alu.mult, op1=Alu.min,
                    )
                    r0 = epi.tile([128, 128], FP32, tag=f"r0_{q % 2}")
                    nc.scalar.activation(
                        out=r0, in_=ps_f, func=Act.Relu,
                        scale=rec_cols[:, q : q + 1],
                    )
                    e0 = epi.tile([128, 128], FP32, tag=f"e0_{q % 2}")
                    nc.scalar.activation(out=e0, in_=m0, func=Act.Exp)
                    nc.vector.scalar_tensor_tensor(
                        out=of_all[:, q, :], in0=e0, scalar=-1.0, in1=r0,
                        op0=Alu.add, op1=Alu.add,
                    )
                nc.sync.dma_start(
                    out=out_d.rearrange("(q p) f -> p q f", p=128), in_=of_all
                )


def kernel(X, A, weight, a, _trace=False, _tmpdir=None):
    X = np.ascontiguousarray(np.asarray(X, dtype=np.float32))
    A = np.ascontiguousarray(np.asarray(A, dtype=np.int32))
    weight = np.ascontiguousarray(np.asarray(weight, dtype=np.float32))
    a = np.ascontiguousarray(np.asarray(a, dtype=np.float32))

    if "nc" not in _cache:
        _cache["nc"] = _build()
    nc = _cache["nc"]

    bf16 = ml_dtypes.bfloat16
    fp8 = ml_dtypes.float8_e4m3

    Xbf = X.astype(bf16)
    # XTg[g, k, p, c] = X[1024 g + c, 128 k + p]
    XTg_base = np.ascontiguousarray(
        Xbf.reshape(NG, ROWS, KC, 128).transpose(0, 2, 3, 1)
    )
    w_bf = weight.astype(bf16)
    ident = np.eye(128, dtype=np.float32)

    in_maps = []
    for c in range(N_CORES):
        i0 = c * ROWS
        # rotate groups so group 0 is this core's own rows; AT8 rows
        # follow the same j-permutation
        perm = [(c + g) % NG for g in range(NG)]
        XTg = np.ascontiguousarray(XTg_base[perm])
        Asl = (A[i0 : i0 + ROWS].astype(np.float32).T * SCALE).astype(fp8)
        AT8 = np.ascontiguousarray(
            Asl.reshape(NG, ROWS, ROWS)[perm].reshape(N, ROWS)
        )
        in_maps.append(
            {
                "XTg": XTg,
                "AT8": AT8,
                "w_in": w_bf,
                "a_vec": a,
                "ident": ident,
            }
        )

    res = run_bass_kernel_spmd(
        nc, in_maps, core_ids=list(range(N_CORES)), trace=_trace, tmpdir=_tmpdir
    )
    out = np.concatenate([res.results[c]["out"] for c in range(N_CORES)], axis=0)
    if _trace:
        kernel._last_results = res
    return out


# revision 50
# speedup vs baseline: 2.5916x; 1.0186x over previous
"""GAT layer (nn_GATLayer) on 8 Trainium2 NeuronCores.

Math (reference):
    Wh = X @ weight                      [N, F]
    s  = Wh @ a[:F];  t = Wh @ a[F:]     [N, 1]
    e  = relu(s_i + t_j)                 [N, N]
    att = softmax(where(A > 0, e, -9e15), axis=1)
    out = elu(att @ Wh)

Kernel formulation (shift-free softmax, exact up to fp rounding):
    p_ij  = A_ij * max(exp(s_i) * exp(t_j), 1)   (exp(relu(x)) = max(exp(x), 1))
    out_i = elu((p_i: @ Wh) / sum_j p_ij)
A global scale c (=1/4) keeps all fp8 operands in e4m3 normal range:
the host mask carries {0, c}, z' = exp(s - ln(1/c)) * exp(t), and every
psum contribution is uniformly c^2-scaled, which cancels in num/den.

Sharding: 1D row partition across 8 cores (1024 rows each). Host-side
prep is layout/dtype only: X^T (bf16, grouped, rotated so group 0 is
own rows), A^T slab per core (fp8 {0, c}, same group rotation), weight
bf16. All model math (Wh, s, t, exp, softmax, aggregation, ELU) runs
on device.

Per-core dataflow, all in [j (partition), i (free)] orientation:
  - setup: w_all[k] = [weight_k | w*a2 | w*a1]; per j-tile pair one PE
    pass gives [Wh | t] (fp8 Wh); s from group-0 X^T; es/s broadcast
    rows via K=1 PE outer products; t transposed to a row for exp.
  - main loop over 32 j-tile pairs, three z sources balanced across
    engines (PE rank-1 outer product into bf16 psum / DVE 4x ptr-mult
    / ACT exp with bias ptr); two mask paths:
      D-pairs: p8 = (c max z) * mask in one DVE op -> 2 DoubleRow
        matmuls (numerator [128 f, 1024 i], denominator [1, 1024]);
      P-pairs: mask-term DR matmuls consume the raw fp8 mask, the
        relu-term r8 = Pool tensor_tensor mult of relu(z - c).
  - epilogue: den -> columns via K=1 matmuls, reciprocal, PE
    transposes to natural [i, f], fused scale+ELU, one output DMA.
"""

import numpy as np
import ml_dtypes

import concourse.bass as bass
import concourse.bacc as bacc
import concourse.mybir as mybir
import concourse.tile as tile
from concourse.bass_utils import run_bass_kernel_spmd

N = 8192
F_IN = 512
F_OUT = 128
N_CORES = 8
ROWS = N // N_CORES          # 1024 rows per core
NT = N // 128                # 64 j tiles
NP = NT // 2                 # 32 j tile pairs
KC = F_IN // 128             # 4 f_in chunks
NG = 8                       # XT groups (8 j-tiles each)

SCALE = 0.25                 # global fp8 range scale c
LNS = float(np.log(1.0 / SCALE))

FP32 = mybir.dt.float32
BF16 = mybir.dt.bfloat16
FP8 = mybir.dt.float8e4
Alu = mybir.AluOpType
Act = mybir.ActivationFunctionType
DR = mybir.MatmulPerfMode.DoubleRow

# --- engine lane tables (tuned against the TimelineSim cost model) ---
# P_PAIRS: pairs routed through the relu decomposition (Pool mask-mult)
P_PAIRS = frozenset({1, 3, 5, 6, 9, 11, 13, 14, 17, 19, 21, 22, 25, 26})
# z source per pair: 'pe' (rank-1 matmul into psum, D-pairs only),
# 'act' (exp), 'dve' (ptr-mult)
_ZSRC = {}
for _t in range(NP):
    _ZSRC[_t] = "dve" if _t in P_PAIRS else "act"
# relu engine for P-pairs: DVE 4x for most, ACT for some
_RELU_ACT = frozenset({3, 11, 19, 27})

_cache = {}


def _build():
    nc = bacc.Bacc("TRN2", target_bir_lowering=False, debug=False,
                   num_devices=N_CORES)

    XTg = nc.dram_tensor("XTg", [NG, KC, 128, ROWS], BF16, kind="ExternalInput")
    AT8 = nc.dram_tensor("AT8", [N, ROWS], FP8, kind="ExternalInput")
    w_in = nc.dram_tensor("w_in", [F_IN, F_OUT], BF16, kind="ExternalInput")
    a_vec = nc.dram_tensor("a_vec", [2 * F_OUT, 1], FP32, kind="ExternalInput")
    ident = nc.dram_tensor("ident", [128, 128], FP32, kind="ExternalInput")
    out_d = nc.dram_tensor("out", [ROWS, F_OUT], FP32, kind="ExternalOutput")

    with tile.TileContext(nc) as tc:
        _body(nc, tc, XTg, AT8, w_in, a_vec, ident, out_d)

    nc.compile()
    return nc


def _body(nc, tc, XTg, AT8, w_in, a_vec, ident, out_d):
    with (
        tc.tile_pool(name="setup", bufs=1) as setup,
        tc.tile_pool(name="xtg", bufs=2) as xtg_pool,
        tc.tile_pool(name="at", bufs=1) as at_pool,
        tc.tile_pool(name="zz", bufs=3) as zz_pool,
        tc.tile_pool(name="pp", bufs=3) as pp_pool,
        tc.tile_pool(name="epi", bufs=1) as epi,
    ):
        setup_psum = tc.tile_pool(name="psA", bufs=1, space="PSUM")
        psA = setup_psum.__enter__()
        # ---------------- setup: weights ----------------
        w_sb = setup.tile([128, KC, 128], BF16)
        nc.sync.dma_start(
            out=w_sb, in_=w_in.rearrange("(k p) f -> p k f", p=128)
        )
        a_cat = setup.tile([128, 2], BF16)
        nc.gpsimd.dma_start(
            out=a_cat, in_=a_vec.rearrange("(h p) o -> p (h o)", p=128)
        )

        # first XT group doubles as own-rows X^T (host rotates groups)
        xtg0 = xtg_pool.tile([128, KC, ROWS], BF16, tag="g0")
        nc.sync.dma_start(out=xtg0, in_=XTg[0].rearrange("k p i -> p k i"))
        idn = setup.tile([128, 128], FP32)
        nc.sync.dma_start(out=idn, in_=ident[:, :])
        idn_bf = setup.tile([128, 128], BF16)
        nc.vector.tensor_copy(idn_bf, idn)

        # w_all[k] = [weight_k | w_t_k | w_s_k]  [128, 130]
        w_all = []
        for k in range(KC):
            wa = setup.tile([128, F_OUT + 2], BF16, tag=f"w_all{k}")
            nc.vector.tensor_copy(wa[:, 0:F_OUT], w_sb[:, k, :])
            ps_wT = psA.tile([128, 128], BF16, tag="wT")
            nc.tensor.transpose(ps_wT, w_sb[:, k, :], idn_bf)
            wT = setup.tile([128, 128], BF16, tag=f"wT{k}")
            nc.vector.tensor_copy(wT, ps_wT)
            ps_a = psA.tile([128, 2], FP32, tag="pa")
            nc.tensor.matmul(ps_a, wT, a_cat, start=True, stop=True)
            # col F_OUT = w_t (a[F:]), col F_OUT+1 = w_s (a[:F])
            nc.vector.tensor_copy(wa[:, F_OUT : F_OUT + 1], ps_a[:, 1:2])
            nc.vector.tensor_copy(wa[:, F_OUT + 1 : F_OUT + 2], ps_a[:, 0:1])
            w_all.append(wa)


        # ---------------- s (own rows) + broadcast rows ----------------
        ps_s = psA.tile([1, ROWS], FP32, tag="ps_s")
        for h in range(2):
            sl = slice(512 * h, 512 * (h + 1))
            for k in range(KC):
                nc.tensor.matmul(
                    ps_s[:, sl],
                    w_all[k][:, F_OUT + 1 : F_OUT + 2],
                    xtg0[:, k, sl],
                    start=(k == 0), stop=(k == KC - 1),
                    skip_group_check=True,
                )
        nls1 = setup.tile([1, 1], FP32, tag="nls1")
        nc.vector.memset(nls1, -LNS)
        es_row = setup.tile([1, ROWS], BF16)
        nc.scalar.activation(out=es_row, in_=ps_s, func=Act.Exp, bias=nls1)
        s_row = setup.tile([1, ROWS], BF16)
        nc.scalar.copy(s_row, ps_s)

        ones_r = setup.tile([1, 128], BF16)
        nc.vector.memset(ones_r, 1.0)
        es_bc = setup.tile([128, ROWS], BF16)
        s_bc = setup.tile([128, ROWS], BF16)
        for h in range(2):
            sl = slice(512 * h, 512 * (h + 1))
            ps_b = psA.tile([128, 512], FP32, tag="bc")
            nc.tensor.matmul(ps_b, ones_r, es_row[:, sl], start=True, stop=True)
            nc.vector.tensor_copy(es_bc[:, sl], ps_b)
            ps_b2 = psA.tile([128, 512], FP32, tag="bc")
            nc.tensor.matmul(ps_b2, ones_r, s_row[:, sl], start=True, stop=True)
            nc.vector.tensor_copy(s_bc[:, sl], ps_b2)

        ones_c = setup.tile([128, 2, 128], FP8)
        nc.vector.memset(ones_c, 1.0)
        ones1 = setup.tile([1, 1], FP32, tag="ones1")
        nc.vector.memset(ones1, 1.0)

        # ---------------- Wh | t for all j tiles ----------------
        wh_all = setup.tile([128, NT, F_OUT], FP8)
        et_cols = setup.tile([128, NT], FP32)
        ts_cols = setup.tile([128, NT], FP32)
        nsc_c = setup.tile([128, 1], FP32, tag="nsc_c")
        nc.vector.memset(nsc_c, -SCALE)
        nsc_ln = setup.tile([128, 1], FP32, tag="nsc_ln")
        nc.vector.memset(nsc_ln, -LNS)

        setup_psum.__exit__(None, None, None)

        with (
            tc.tile_pool(name="psO", bufs=1, space="PSUM") as psO,
            tc.tile_pool(name="psD", bufs=1, space="PSUM") as psD,
        ):
            ps_oT = psO.tile([128, ROWS], FP32)
            ps_d = psD.tile([128, ROWS], FP32)

            with tc.tile_pool(name="psS", bufs=4, space="PSUM") as psS:
                at_tiles = []

                def emit_setup(g):
                    if g == 0:
                        xtg = xtg0
                    else:
                        xtg = xtg_pool.tile([128, KC, ROWS], BF16)
                        nc.sync.dma_start(
                            out=xtg, in_=XTg[g].rearrange("k p i -> p k i")
                        )
                    at = at_pool.tile([128, 8, ROWS], FP8, tag=f"at{g}")
                    at_tiles.append(at)
                    nc.sync.dma_start(
                        out=at,
                        in_=AT8[ROWS * g : ROWS * (g + 1), :].rearrange(
                            "(t p) i -> p t i", p=128
                        ),
                    )
                    for q in range(4):
                        jt0 = 8 * g + 2 * q
                        ps_p = psS.tile([128, 2, F_OUT + 2], FP32)
                        for v in range(2):
                            co = 128 * (2 * q + v)
                            for k in range(KC):
                                nc.tensor.matmul(
                                    ps_p[:, v, :],
                                    xtg[:, k, co : co + 128],
                                    w_all[k],
                                    start=(k == 0), stop=(k == KC - 1),
                                    skip_group_check=True,
                                )
                        # copies (gpsimd cannot access PSUM)
                        nc.scalar.copy(
                            wh_all[:, jt0 : jt0 + 2, :], ps_p[:, :, 0:F_OUT]
                        )
                        nc.scalar.activation(
                            out=et_cols[:, jt0 : jt0 + 2],
                            in_=ps_p[:, :, F_OUT : F_OUT + 1],
                            func=Act.Exp,
                        )
                        nc.vector.tensor_scalar(
                            out=ts_cols[:, jt0 : jt0 + 2],
                            in0=ps_p[:, :, F_OUT : F_OUT + 1],
                            scalar1=-LNS, scalar2=None, op0=Alu.add,
                        )

                def emit_pair(t):
                    at = at_tiles[t // 4]
                    s0 = 2 * (t % 4)
                    atsl = at[:, s0 : s0 + 2, :]
                    first, last = t == 0, t == NP - 1
                    zp = zz_pool.tile([128, 2, ROWS], BF16)
                    for v in range(2):
                        jt = 2 * t + v
                        if _ZSRC[t] == "act":
                            nc.scalar.activation(
                                out=zp[:, v, :], in_=s_bc, func=Act.Exp,
                                bias=ts_cols[:, jt : jt + 1],
                            )
                        else:
                            nc.vector.tensor_scalar(
                                out=zp[:, v, :], in0=es_bc,
                                scalar1=et_cols[:, jt : jt + 1],
                                scalar2=None, op0=Alu.mult,
                            )
                    if t in P_PAIRS:
                        # clamp on DVE (4x), mask-mult on Pool: the host
                        # mask already carries the scale c
                        rt = pp_pool.tile([128, 2, ROWS], BF16, tag="rt")
                        nc.vector.tensor_scalar(
                            out=rt, in0=zp, scalar1=SCALE, scalar2=None,
                            op0=Alu.max,
                        )
                        pp = pp_pool.tile([128, 2, ROWS], FP8, tag="p8")
                        nc.gpsimd.tensor_tensor(
                            out=pp, in0=rt, in1=atsl, op=Alu.mult
                        )
                    else:
                        pp = pp_pool.tile([128, 2, ROWS], FP8, tag="p8")
                        nc.vector.scalar_tensor_tensor(
                            out=pp, in0=zp, scalar=SCALE,
                            in1=atsl, op0=Alu.max, op1=Alu.mult,
                        )
                    if True:
                        for h in range(2):
                            sl = slice(512 * h, 512 * (h + 1))
                            nc.tensor.matmul(
                                ps_oT[:, sl],
                                wh_all[:, 2 * t : 2 * t + 2, :],
                                pp[:, :, sl], start=first, stop=last,
                                perf_mode=DR, skip_group_check=True,
                            )
                            nc.tensor.matmul(
                                ps_d[:, sl], ones_c, pp[:, :, sl],
                                start=first, stop=last,
                                perf_mode=DR, skip_group_check=True,
                            )

                # software-pipelined emission: setup(g+1) ahead of pairs(g)
                emit_setup(0)
                for g in range(NG):
                    if g + 1 < NG:
                        emit_setup(g + 1)
                    for t in range(4 * g, 4 * g + 4):
                        emit_pair(t)

            # ---------------- epilogue ----------------
            with tc.tile_pool(name="psE", bufs=2, space="PSUM") as psE:
                den_row = epi.tile([1, ROWS], FP32, tag="den")
                nc.scalar.copy(den_row, ps_d[0:1, :])
                ps_dc = psE.tile([128, 8], FP32, tag="dc")
                for q in range(8):
                    nc.tensor.matmul(
                        ps_dc[:, q : q + 1],
                        den_row[:, 128 * q : 128 * (q + 1)], ones1,
                        start=True, stop=True, skip_group_check=True,
                    )
                rec_cols = epi.tile([128, 8], FP32, tag="rec")
                nc.vector.reciprocal(rec_cols, ps_dc)
                num_sb = epi.tile([128, ROWS], FP32, tag="num")
                for q in range(8):
                    qs = slice(128 * q, 128 * (q + 1))
                    nc.vector.tensor_copy(num_sb[:, qs], ps_oT[:, qs])
                of_all = epi.tile([128, 8, F_OUT], FP32, tag="of")
                for q in range(8):
                    ps_f = psE.tile([128, 128], FP32, tag="f")
                    nc.tensor.transpose(
                        ps_f, num_sb[:, 128 * q : 128 * (q + 1)], idn
                    )
                    m0 = epi.tile([128, 128], FP32, tag=f"m0_{q % 2}")
                    nc.vector.tensor_scalar(
                        out=m0, in0=ps_f, scalar1=rec_cols[:, q : q + 1],
                        scalar2=0.0, op0=Alu.mult, op1=Alu.min,
                    )
                    r0 = epi.tile([128, 128], FP32, tag=f"r0_{q % 2}")
                    nc.scalar.activation(
                        out=r0, in_=ps_f, func=Act.Relu,
                        scale=rec_cols[:, q : q + 1],
                    )
                    e0 = epi.tile([128, 128], FP32, tag=f"e0_{q % 2}")
                    nc.scalar.activation(out=e0, in_=m0, func=Act.Exp)
                    nc.vector.scalar_tensor_tensor(
                        out=of_all[:, q, :], in0=e0, scalar=-1.0, in1=r0,
                        op0=Alu.add, op1=Alu.add,
                    )
                    if q == 3:
                        nc.sync.dma_start(
                            out=out_d[0 : 4 * 128, :].rearrange(
                                "(q p) f -> p q f", p=128
                            ),
                            in_=of_all[:, 0:4, :],
                        )
                nc.sync.dma_start(
                    out=out_d[4 * 128 : 8 * 128, :].rearrange(
                        "(q p) f -> p q f", p=128
                    ),
                    in_=of_all[:, 4:8, :],
                )


def kernel(X, A, weight, a, _trace=False, _tmpdir=None):
    X = np.ascontiguousarray(np.asarray(X, dtype=np.float32))
    A = np.ascontiguousarray(np.asarray(A, dtype=np.int32))
    weight = np.ascontiguousarray(np.asarray(weight, dtype=np.float32))
    a = np.ascontiguousarray(np.asarray(a, dtype=np.float32))

    if "nc" not in _cache:
        _cache["nc"] = _build()
    nc = _cache["nc"]

    bf16 = ml_dtypes.bfloat16
    fp8 = ml_dtypes.float8_e4m3

    Xbf = X.astype(bf16)
    # XTg[g, k, p, c] = X[1024 g + c, 128 k + p]
    XTg_base = np.ascontiguousarray(
        Xbf.reshape(NG, ROWS, KC, 128).transpose(0, 2, 3, 1)
    )
    w_bf = weight.astype(bf16)
    ident = np.eye(128, dtype=np.float32)

    in_maps = []
    for c in range(N_CORES):
        i0 = c * ROWS
        # rotate groups so group 0 is this core's own rows; AT8 rows
        # follow the same j-permutation
        perm = [(c + g) % NG for g in range(NG)]
        XTg = np.ascontiguousarray(XTg_base[perm])
        Asl = (A[i0 : i0 + ROWS].astype(np.float32).T * SCALE).astype(fp8)
        AT8 = np.ascontiguousarray(
            Asl.reshape(NG, ROWS, ROWS)[perm].reshape(N, ROWS)
        )
        in_maps.append(
            {
                "XTg": XTg,
                "AT8": AT8,
                "w_in": w_bf,
                "a_vec": a,
                "ident": ident,
            }
        )

    res = run_bass_kernel_spmd(
        nc, in_maps, core_ids=list(range(N_CORES)), trace=_trace, tmpdir=_tmpdir
    )
    out = np.concatenate([res.results[c]["out"] for c in range(N_CORES)], axis=0)
    if _trace:
        kernel._last_results = res
    return out


# revision 60
# speedup vs baseline: 2.9781x; 1.1491x over previous
"""GAT layer (nn_GATLayer) on 8 Trainium2 NeuronCores.

Math (reference):
    Wh = X @ weight                      [N, F]
    s  = Wh @ a[:F];  t = Wh @ a[F:]     [N, 1]
    e  = relu(s_i + t_j)                 [N, N]
    att = softmax(where(A > 0, e, -9e15), axis=1)
    out = elu(att @ Wh)

Kernel formulation (shift-free softmax, exact up to fp rounding):
    p_ij  = A_ij * max(exp(s_i) * exp(t_j), 1)   (exp(relu(x)) = max(exp(x), 1))
    out_i = elu((p_i: @ Wh) / sum_j p_ij)
A global scale c (=1/4) keeps all fp8 operands in e4m3 normal range:
the host mask carries {0, c}, z' = exp(s - ln(1/c)) * exp(t), and every
psum contribution is uniformly c^2-scaled, which cancels in num/den.

Sharding: 1D row partition across 8 cores (1024 rows each). Host-side
prep is layout/dtype only: X^T (bf16, grouped, rotated so group 0 is
own rows), A^T slab per core (fp8 {0, c}, same group rotation), weight
bf16. All model math (Wh, s, t, exp, softmax, aggregation, ELU) runs
on device.

Per-core dataflow, all in [j (partition), i (free)] orientation:
  - setup: w_all[k] = [weight_k | w*a2 | w*a1]; per j-tile pair one PE
    pass gives [Wh | t] (fp8 Wh); s from group-0 X^T; es/s broadcast
    rows via K=1 PE outer products; t transposed to a row for exp.
  - main loop over 32 j-tile pairs, three z sources balanced across
    engines (PE rank-1 outer product into bf16 psum / DVE 4x ptr-mult
    / ACT exp with bias ptr); two mask paths:
      D-pairs: p8 = (c max z) * mask in one DVE op -> 2 DoubleRow
        matmuls (numerator [128 f, 1024 i], denominator [1, 1024]);
      P-pairs: mask-term DR matmuls consume the raw fp8 mask, the
        relu-term r8 = Pool tensor_tensor mult of relu(z - c).
  - epilogue: den -> columns via K=1 matmuls, reciprocal, PE
    transposes to natural [i, f], fused scale+ELU, one output DMA.
"""

import numpy as np
import ml_dtypes

import concourse.bass as bass
import concourse.bacc as bacc
import concourse.mybir as mybir
import concourse.tile as tile
from concourse.bass_utils import run_bass_kernel_spmd

N = 8192
F_IN = 512
F_OUT = 128
N_CORES = 8
ROWS = N // N_CORES          # 1024 rows per core
NT = N // 128                # 64 j tiles
NP = NT // 2                 # 32 j tile pairs
KC = F_IN // 128             # 4 f_in chunks
NG = 8                       # XT groups (8 j-tiles each)

SCALE = 0.25                 # global fp8 range scale c
LNS = float(np.log(1.0 / SCALE))

FP32 = mybir.dt.float32
BF16 = mybir.dt.bfloat16
FP8 = mybir.dt.float8e4
Alu = mybir.AluOpType
Act = mybir.ActivationFunctionType
DR = mybir.MatmulPerfMode.DoubleRow

# --- engine lane tables (tuned against the TimelineSim cost model) ---
# P_PAIRS: pairs routed through the relu decomposition (Pool mask-mult)
P_PAIRS = frozenset({1, 3, 5, 6, 9, 11, 13, 14, 17, 19, 21, 22, 25, 26})
# z source per pair: 'pe' (rank-1 matmul into psum, D-pairs only),
# 'act' (exp), 'dve' (ptr-mult)
_ZSRC = {}
for _t in range(NP):
    _ZSRC[_t] = "dve" if _t in P_PAIRS else "act"
# relu engine for P-pairs: DVE 4x for most, ACT for some
_RELU_ACT = frozenset({3, 11, 19, 27})

_cache = {}


def _build():
    nc = bacc.Bacc("TRN2", target_bir_lowering=False, debug=False,
                   num_devices=N_CORES)

    XTg = nc.dram_tensor("XTg", [NG, KC, 128, ROWS], BF16, kind="ExternalInput")
    AT8 = nc.dram_tensor("AT8", [N, ROWS], FP8, kind="ExternalInput")
    w_in = nc.dram_tensor("w_in", [F_IN, F_OUT], BF16, kind="ExternalInput")
    a_vec = nc.dram_tensor("a_vec", [2 * F_OUT, 1], FP32, kind="ExternalInput")
    ident = nc.dram_tensor("ident", [128, 128], FP32, kind="ExternalInput")
    out_d = nc.dram_tensor("out", [ROWS, F_OUT], FP32, kind="ExternalOutput")

    with tile.TileContext(nc) as tc:
        _body(nc, tc, XTg, AT8, w_in, a_vec, ident, out_d)

    nc.compile()
    return nc


def _body(nc, tc, XTg, AT8, w_in, a_vec, ident, out_d):
    with (
        tc.tile_pool(name="setup", bufs=1) as setup,
        tc.tile_pool(name="xtg", bufs=2) as xtg_pool,
        tc.tile_pool(name="at", bufs=1) as at_pool,
        tc.tile_pool(name="zz", bufs=5) as zz_pool,
        tc.tile_pool(name="pp", bufs=6) as pp_pool,
        tc.tile_pool(name="epi", bufs=1) as epi,
    ):
        setup_psum = tc.tile_pool(name="psA", bufs=1, space="PSUM")
        psA = setup_psum.__enter__()
        # ---------------- setup: weights ----------------
        w_sb = setup.tile([128, KC, 128], BF16)
        nc.sync.dma_start(
            out=w_sb, in_=w_in.rearrange("(k p) f -> p k f", p=128)
        )
        idn = setup.tile([128, 128], FP32)
        nc.sync.dma_start(out=idn, in_=ident[:, :])
        a_cat = setup.tile([128, 2], BF16)
        nc.gpsimd.dma_start(
            out=a_cat, in_=a_vec.rearrange("(h p) o -> p (h o)", p=128)
        )

        # first XT group doubles as own-rows X^T (host rotates groups)
        xtg0 = xtg_pool.tile([128, KC, ROWS], BF16, tag="g0")
        nc.sync.dma_start(out=xtg0, in_=XTg[0].rearrange("k p i -> p k i"))
        idn_bf = setup.tile([128, 128], BF16)
        nc.vector.tensor_copy(idn_bf, idn)

        # w_all[k] = [weight_k | w_t_k | w_s_k]  [128, 130]
        w_all = []
        for k in range(KC):
            wa = setup.tile([128, F_OUT + 2], BF16, tag=f"w_all{k}")
            nc.vector.tensor_copy(wa[:, 0:F_OUT], w_sb[:, k, :])
            ps_wT = psA.tile([128, 128], BF16, tag="wT")
            nc.tensor.transpose(ps_wT, w_sb[:, k, :], idn_bf)
            wT = setup.tile([128, 128], BF16, tag=f"wT{k}")
            nc.vector.tensor_copy(wT, ps_wT)
            ps_a = psA.tile([128, 2], FP32, tag="pa")
            nc.tensor.matmul(ps_a, wT, a_cat, start=True, stop=True)
            # col F_OUT = w_t (a[F:]), col F_OUT+1 = w_s (a[:F])
            nc.vector.tensor_copy(wa[:, F_OUT : F_OUT + 1], ps_a[:, 1:2])
            nc.vector.tensor_copy(wa[:, F_OUT + 1 : F_OUT + 2], ps_a[:, 0:1])
            w_all.append(wa)


        # ---------------- s (own rows) + broadcast rows ----------------
        ps_s = psA.tile([1, ROWS], FP32, tag="ps_s")
        for h in range(2):
            sl = slice(512 * h, 512 * (h + 1))
            for k in range(KC):
                nc.tensor.matmul(
                    ps_s[:, sl],
                    w_all[k][:, F_OUT + 1 : F_OUT + 2],
                    xtg0[:, k, sl],
                    start=(k == 0), stop=(k == KC - 1),
                    skip_group_check=True,
                )
        nls1 = setup.tile([1, 1], FP32, tag="nls1")
        nc.vector.memset(nls1, -LNS)
        es_row = setup.tile([1, ROWS], BF16)
        nc.scalar.activation(out=es_row, in_=ps_s, func=Act.Exp, bias=nls1)
        s_row = setup.tile([1, ROWS], BF16)
        nc.scalar.copy(s_row, ps_s)

        ones_r = setup.tile([1, 128], BF16)
        nc.vector.memset(ones_r, 1.0)
        es_bc = setup.tile([128, ROWS], BF16)
        s_bc = setup.tile([128, ROWS], BF16)
        for h in range(2):
            sl = slice(512 * h, 512 * (h + 1))
            ps_b = psA.tile([128, 512], FP32, tag="bc", bufs=2)
            nc.tensor.matmul(ps_b, ones_r, es_row[:, sl], start=True, stop=True)
            nc.vector.tensor_copy(es_bc[:, sl], ps_b)
            ps_b2 = psA.tile([128, 512], FP32, tag="bc", bufs=2)
            nc.tensor.matmul(ps_b2, ones_r, s_row[:, sl], start=True, stop=True)
            nc.vector.tensor_copy(s_bc[:, sl], ps_b2)

        ones_c = setup.tile([128, 2, 128], FP8)
        nc.vector.memset(ones_c, 1.0)
        ones1 = setup.tile([1, 1], FP32, tag="ones1")
        nc.vector.memset(ones1, 1.0)

        # ---------------- Wh | t for all j tiles ----------------
        wh_all = setup.tile([128, NT, F_OUT], FP8)
        et_cols = setup.tile([128, NT], FP32)
        ts_cols = setup.tile([128, NT], FP32)
        nsc_c = setup.tile([128, 1], FP32, tag="nsc_c")
        nc.vector.memset(nsc_c, -SCALE)
        nsc_ln = setup.tile([128, 1], FP32, tag="nsc_ln")
        nc.vector.memset(nsc_ln, -LNS)

        setup_psum.__exit__(None, None, None)

        with (
            tc.tile_pool(name="psO", bufs=1, space="PSUM") as psO,
            tc.tile_pool(name="psD", bufs=1, space="PSUM") as psD,
        ):
            ps_oT = psO.tile([128, ROWS], FP32)
            ps_d = psD.tile([128, ROWS], FP32)

            with tc.tile_pool(name="psS", bufs=4, space="PSUM") as psS:
                at_tiles = []

                def emit_setup(g):
                    if g == 0:
                        xtg = xtg0
                    else:
                        xtg = xtg_pool.tile([128, KC, ROWS], BF16)
                        nc.sync.dma_start(
                            out=xtg, in_=XTg[g].rearrange("k p i -> p k i")
                        )
                    at = at_pool.tile([128, 8, ROWS], FP8, tag=f"at{g}")
                    at_tiles.append(at)
                    nc.sync.dma_start(
                        out=at,
                        in_=AT8[ROWS * g : ROWS * (g + 1), :].rearrange(
                            "(t p) i -> p t i", p=128
                        ),
                    )
                    for q in range(4):
                        jt0 = 8 * g + 2 * q
                        ps_p = psS.tile([128, 2, F_OUT + 2], FP32)
                        for v in range(2):
                            co = 128 * (2 * q + v)
                            for k in range(KC):
                                nc.tensor.matmul(
                                    ps_p[:, v, :],
                                    xtg[:, k, co : co + 128],
                                    w_all[k],
                                    start=(k == 0), stop=(k == KC - 1),
                                    skip_group_check=True,
                                )
                        # copies (gpsimd cannot access PSUM)
                        nc.scalar.copy(
                            wh_all[:, jt0 : jt0 + 2, :], ps_p[:, :, 0:F_OUT]
                        )
                        nc.scalar.activation(
                            out=et_cols[:, jt0 : jt0 + 2],
                            in_=ps_p[:, :, F_OUT : F_OUT + 1],
                            func=Act.Exp,
                        )
                        nc.vector.tensor_scalar(
                            out=ts_cols[:, jt0 : jt0 + 2],
                            in0=ps_p[:, :, F_OUT : F_OUT + 1],
                            scalar1=-LNS, scalar2=None, op0=Alu.add,
                        )

                z_early = {}

                def emit_z(t, zp):
                    for v in range(2):
                        jt = 2 * t + v
                        if _ZSRC[t] == "act":
                            nc.scalar.activation(
                                out=zp[:, v, :], in_=s_bc, func=Act.Exp,
                                bias=ts_cols[:, jt : jt + 1],
                            )
                        else:
                            nc.vector.tensor_scalar(
                                out=zp[:, v, :], in0=es_bc,
                                scalar1=et_cols[:, jt : jt + 1],
                                scalar2=None, op0=Alu.mult,
                            )

                def emit_pair(t):
                    at = at_tiles[t // 4]
                    s0 = 2 * (t % 4)
                    atsl = at[:, s0 : s0 + 2, :]
                    first, last = t == 0, t == NP - 1
                    if t in z_early:
                        zp = z_early[t]
                    else:
                        zp = zz_pool.tile([128, 2, ROWS], BF16)
                        emit_z(t, zp)
                    if t in P_PAIRS:
                        # clamp on DVE (4x), mask-mult on Pool: the host
                        # mask already carries the scale c
                        rt = pp_pool.tile([128, 2, ROWS], BF16, tag="rt")
                        nc.vector.tensor_scalar(
                            out=rt, in0=zp, scalar1=SCALE, scalar2=None,
                            op0=Alu.max,
                        )
                        pp = pp_pool.tile([128, 2, ROWS], FP8, tag="p8")
                        nc.gpsimd.tensor_tensor(
                            out=pp, in0=rt, in1=atsl, op=Alu.mult
                        )
                    else:
                        pp = pp_pool.tile([128, 2, ROWS], FP8, tag="p8")
                        nc.vector.scalar_tensor_tensor(
                            out=pp, in0=zp, scalar=SCALE,
                            in1=atsl, op0=Alu.max, op1=Alu.mult,
                        )
                    if True:
                        for h in range(2):
                            sl = slice(512 * h, 512 * (h + 1))
                            nc.tensor.matmul(
                                ps_oT[:, sl],
                                wh_all[:, 2 * t : 2 * t + 2, :],
                                pp[:, :, sl], start=first, stop=last,
                                perf_mode=DR, skip_group_check=True,
                            )
                            nc.tensor.matmul(
                                ps_d[:, sl], ones_c, pp[:, :, sl],
                                start=first, stop=last,
                                perf_mode=DR, skip_group_check=True,
                            )

                # software-pipelined emission: setup(g+1) ahead of pairs(g);
                # z ops for group 0's pairs go ahead of group 1's copies
                emit_setup(0)
                for _te in range(4):
                    zpe = zz_pool.tile([128, 2, ROWS], BF16, tag=f"zpe{_te}", bufs=1, name=f"zpe{_te}")
                    z_early[_te] = zpe
                    emit_z(_te, zpe)
                for g in range(NG):
                    if g + 1 < NG:
                        emit_setup(g + 1)
                    for t in range(4 * g, 4 * g + 4):
                        emit_pair(t)

            # ---------------- epilogue ----------------
            with tc.tile_pool(name="psE", bufs=2, space="PSUM") as psE:
                den_row = epi.tile([1, ROWS], FP32, tag="den")
                nc.scalar.copy(den_row, ps_d[0:1, :])
                ps_dc = psE.tile([128, 8], FP32, tag="dc")
                for q in range(8):
                    nc.tensor.matmul(
                        ps_dc[:, q : q + 1],
                        den_row[:, 128 * q : 128 * (q + 1)], ones1,
                        start=True, stop=True, skip_group_check=True,
                    )
                rec_cols = epi.tile([128, 8], FP32, tag="rec")
                nc.vector.reciprocal(rec_cols, ps_dc)
                num_sb = epi.tile([128, ROWS], FP32, tag="num")
                for q in range(8):
                    qs = slice(128 * q, 128 * (q + 1))
                    nc.vector.tensor_copy(num_sb[:, qs], ps_oT[:, qs])
                of_all = epi.tile([128, 8, F_OUT], FP32, tag="of")
                for q in range(8):
                    ps_f = psE.tile([128, 128], FP32, tag="f")
                    nc.tensor.transpose(
                        ps_f, num_sb[:, 128 * q : 128 * (q + 1)], idn
                    )
                    m0 = epi.tile([128, 128], FP32, tag=f"m0_{q % 2}")
                    nc.vector.tensor_scalar(
                        out=m0, in0=ps_f, scalar1=rec_cols[:, q : q + 1],
                        scalar2=0.0, op0=Alu.mult, op1=Alu.min,
                    )
                    r0 = epi.tile([128, 128], FP32, tag=f"r0_{q % 2}")
                    nc.scalar.activation(
                        out=r0, in_=ps_f, func=Act.Relu,
                        scale=rec_cols[:, q : q + 1],
                    )
                    e0 = epi.tile([128, 128], FP32, tag=f"e0_{q % 2}")
                    nc.scalar.activation(out=e0, in_=m0, func=Act.Exp)
                    nc.vector.scalar_tensor_tensor(
                        out=of_all[:, q, :], in0=e0, scalar=-1.0, in1=r0,
                        op0=Alu.add, op1=Alu.add,
                    )
                    if q == 3:
                        nc.sync.dma_start(
                            out=out_d[0 : 4 * 128, :].rearrange(
                                "(q p) f -> p q f", p=128
                            ),
                            in_=of_all[:, 0:4, :],
                        )
                nc.sync.dma_start(
                    out=out_d[4 * 128 : 8 * 128, :].rearrange(
                        "(q p) f -> p q f", p=128
                    ),
                    in_=of_all[:, 4:8, :],
                )


def kernel(X, A, weight, a, _trace=False, _tmpdir=None):
    X = np.ascontiguousarray(np.asarray(X, dtype=np.float32))
    A = np.ascontiguousarray(np.asarray(A, dtype=np.int32))
    weight = np.ascontiguousarray(np.asarray(weight, dtype=np.float32))
    a = np.ascontiguousarray(np.asarray(a, dtype=np.float32))

    if "nc" not in _cache:
        _cache["nc"] = _build()
    nc = _cache["nc"]

    bf16 = ml_dtypes.bfloat16
    fp8 = ml_dtypes.float8_e4m3

    Xbf = X.astype(bf16)
    # XTg[g, k, p, c] = X[1024 g + c, 128 k + p]
    XTg_base = np.ascontiguousarray(
        Xbf.reshape(NG, ROWS, KC, 128).transpose(0, 2, 3, 1)
    )
    w_bf = weight.astype(bf16)
    ident = np.eye(128, dtype=np.float32)

    in_maps = []
    for c in range(N_CORES):
        i0 = c * ROWS
        # rotate groups so group 0 is this core's own rows; AT8 rows
        # follow the same j-permutation
        perm = [(c + g) % NG for g in range(NG)]
        XTg = np.ascontiguousarray(XTg_base[perm])
        Asl = (A[i0 : i0 + ROWS].astype(np.float32).T * SCALE).astype(fp8)
        AT8 = np.ascontiguousarray(
            Asl.reshape(NG, ROWS, ROWS)[perm].reshape(N, ROWS)
        )
        in_maps.append(
            {
                "XTg": XTg,
                "AT8": AT8,
                "w_in": w_bf,
                "a_vec": a,
                "ident": ident,
            }
        )

    res = run_bass_kernel_spmd(
        nc, in_maps, core_ids=list(range(N_CORES)), trace=_trace, tmpdir=_tmpdir
    )
    out = np.concatenate([res.results[c]["out"] for c in range(N_CORES)], axis=0)
    if _trace:
        kernel._last_results = res
    return out
